# revision 5
# baseline (speedup 1.0000x reference)
"""Trainium2 Bass kernel for nn_ExplicitInterClassGraphLoss.

Reference computes:
    zg[a,b] = clip(cos(|z_a - z_b|), 0, 1),  z_i = i * (pi/2) / 39
    lbl_adj = zg[label_i, label_j]                      # [bs, bs]
    fn      = feat / max(||feat_row||, 1e-8)
    loss    = mean((fn @ fn.T - lbl_adj)^2)

Since z in [0, pi/2], |z_a - z_b| <= pi/2 so the clip is a no-op and
    zg[a,b] = cos(z_a)cos(z_b) + sin(z_a)sin(z_b)   (rank 2).
With A = [fn, c, s] (bs x 514) and W = diag(1...1, -1, -1):
    diff = A W A^T,  and
    sum(diff^2) = sum_{kl} W_k W_l (A^T A)_{kl}^2
                = S_ff + S_cc - 2 * S_fc   over the blocks of P = A^T A.
So the 8192x8192 Gram never needs to be materialized: each core computes a
partial P_m = A_m^T A_m over its 1024 batch rows (contraction over batch =
rows are PE partitions, no transposes needed), P is all-reduced across the
8 cores, and every core computes the weighted sum of squares of P.

Sharding: data-parallel over batch rows (1024 rows/core). Output of the
kernel is the final scalar loss (weights carry the 1/bs^2factor); the host
just reads core 0's scalar.
"""

import math

import numpy as np

N_CORES = 8
BS = 8192
D = 512
LOCAL = BS // N_CORES  # 1024
NT = LOCAL // 128  # 8 tiles of 128 batch rows per core
KA = D + 2  # 514 = features + cos + sin columns
PAD = D + 4  # 516: c,s cols + 2 cols holding the 2x2 label-Gram block
NUM_BINS = 40
DELTA = math.pi / 2.0 / (NUM_BINS - 1)
EPS = 1e-8

# Set False to skip the on-device AllReduce (host then sums the per-core
# partial P matrices; used for bring-up/debug).
USE_COLLECTIVE = True

_CACHE = {}


def _build_module(use_collective):
    import concourse.bass as bass  # noqa: F401
    import concourse.tile as tile
    from concourse import bacc, mybir

    f32 = mybir.dt.float32
    f32r = mybir.dt.float32r
    AF = mybir.ActivationFunctionType
    ALU = mybir.AluOpType

    nc = bacc.Bacc(
        "TRN2",
        target_bir_lowering=False,
        debug=False,
        enable_asserts=False,
        num_devices=N_CORES,
    )

    feat_loc = nc.dram_tensor("feat_loc", [LOCAL, D], f32, kind="ExternalInput").ap()
    lab_grid = nc.dram_tensor("lab_grid", [128, NT], f32, kind="ExternalInput").ap()
    wvec_in = nc.dram_tensor("wvec", [1, 16], f32, kind="ExternalInput").ap()
    out_num = nc.dram_tensor("out_num", [1, 1], f32, kind="ExternalOutput").ap()
    out_p = nc.dram_tensor("out_p", [512, PAD], f32, kind="ExternalOutput").ap()

    with tile.TileContext(nc) as tc:
        with (
            tc.tile_pool(name="apool", bufs=NT) as apool,
            tc.tile_pool(name="scratch", bufs=2) as scratch,
            tc.tile_pool(name="small", bufs=4) as small,
            tc.tile_pool(name="res", bufs=1) as res,
            tc.tile_pool(name="work", bufs=2) as work,
            tc.tile_pool(name="pmain", bufs=2, space="PSUM") as pmain,
            tc.tile_pool(name="pside", bufs=2, space="PSUM") as pside,
            tc.tile_pool(name="pfin", bufs=1, space="PSUM") as pfin,
            tc.tile_pool(name="dram", bufs=1, space="DRAM") as dram,
        ):
            # ---- load inputs -------------------------------------------------
            labs = res.tile([128, NT], f32, name="labs")
            nc.sync.dma_start(out=labs, in_=lab_grid)
            pihalf = res.tile([128, 1], f32, name="pihalf")
            nc.vector.memset(pihalf, math.pi / 2.0)
            wv = res.tile([1, 16], f32, name="wv")
            nc.sync.dma_start(out=wv, in_=wvec_in)

            a_tiles = []
            for t in range(NT):
                at = apool.tile([128, KA], f32, name=f"a{t}", tag="A")
                nc.sync.dma_start(
                    out=at[:, 0:D], in_=feat_loc[t * 128 : (t + 1) * 128, :]
                )
                a_tiles.append(at)

            # ---- row norms: ss = sum(feat^2) along features ------------------
            ss_tiles = []
            for t in range(NT):
                sq = scratch.tile([128, D], f32, name=f"sq{t}", tag="sq")
                ss = small.tile([128, 1], f32, name=f"ss{t}", tag=f"ss{t}")
                nc.scalar.activation(
                    out=sq, in_=a_tiles[t][:, 0:D], func=AF.Square, accum_out=ss
                )
                ss_tiles.append(ss)

            # w0 = sqrt(ss) via ACT (sqrt_and_others table set, batched)
            w_tiles = []
            for t in range(NT):
                w0 = small.tile([128, 1], f32, name=f"w{t}", tag=f"w{t}")
                nc.scalar.activation(out=w0, in_=ss_tiles[t], func=AF.Sqrt)
                w_tiles.append(w0)

            # r = 1/max(w0, eps), then one Newton step on rsqrt to wash out
            # the ACT sqrt table error: r <- r * (1.5 - 0.5 * ss * r^2)
            for t in range(NT):
                w0 = w_tiles[t]
                nc.vector.tensor_scalar_max(out=w0, in0=w0, scalar1=EPS)
                nc.vector.reciprocal(out=w0, in_=w0)
                t1 = small.tile([128, 1], f32, name=f"t1_{t}", tag="t1")
                nc.vector.tensor_mul(t1, w0, w0)
                nc.vector.tensor_mul(t1, t1, ss_tiles[t])
                nc.vector.tensor_scalar(
                    out=t1, in0=t1, scalar1=-0.5, scalar2=1.5, op0=ALU.mult, op1=ALU.add
                )
                nc.vector.tensor_mul(w0, w0, t1)
                # normalize the feature block in place
                nc.vector.tensor_scalar_mul(
                    out=a_tiles[t][:, 0:D], in0=a_tiles[t][:, 0:D], scalar1=w0
                )

            # ---- c/s columns: cos(z) = sin(z + pi/2), z = label * DELTA ------
            for t in range(NT):
                at = a_tiles[t]
                nc.scalar.activation(
                    out=at[:, D : D + 1],
                    in_=labs[:, t : t + 1],
                    func=AF.Sin,
                    bias=pihalf[:, 0:1],
                    scale=DELTA,
                )
                nc.scalar.activation(
                    out=at[:, D + 1 : D + 2],
                    in_=labs[:, t : t + 1],
                    func=AF.Sin,
                    bias=0.0,
                    scale=DELTA,
                )

            # ---- f32r shadow copies of A (FP32r matmul operands must be
            # produced by an instruction that rounds to FP32r) --------------
            ar_tiles = []
            for t in range(NT):
                ar = apool.tile([128, KA], f32r, name=f"ar{t}", tag="Ar")
                nc.vector.tensor_copy(out=ar, in_=a_tiles[t])
                ar_tiles.append(ar)

            # ---- Gram P_m = A^T A (contract over the 1024 local batch rows) --
            # ff + fc blocks in float32r (full PE rate), cc block in exact f32.
            psb_tiles = []
            for mt in range(4):
                pm = pmain.tile([128, D], f32, name=f"pm{mt}", tag="pm")
                ps2 = pside.tile([128, 2], f32, name=f"ps2_{mt}", tag="ps2")
                for k in range(NT):
                    ak = ar_tiles[k]
                    lhs = ak[:, mt * 128 : (mt + 1) * 128]
                    nc.tensor.matmul(
                        pm,
                        lhsT=lhs,
                        rhs=ak[:, 0:D],
                        start=(k == 0),
                        stop=(k == NT - 1),
                    )
                    nc.tensor.matmul(
                        ps2,
                        lhsT=lhs,
                        rhs=ak[:, D : D + 2],
                        start=(k == 0),
                        stop=(k == NT - 1),
                    )
                psb = res.tile([128, PAD], f32, name=f"psb{mt}", tag=f"psb{mt}")
                nc.vector.tensor_copy(out=psb[:, 0:D], in_=pm)
                nc.vector.tensor_copy(out=psb[:, D : D + 2], in_=ps2)
                nc.vector.memset(psb[:, D + 2 : PAD], 0.0)
                psb_tiles.append(psb)

            # 2x2 label-Gram block [a d; d b], exact f32, stored in cols
            # D+2:D+4 of the first row-block.
            p22 = pside.tile([2, 2], f32, name="p22", tag="p22")
            for k in range(NT):
                ak = a_tiles[k]
                nc.tensor.matmul(
                    p22,
                    lhsT=ak[:, D : D + 2],
                    rhs=ak[:, D : D + 2],
                    start=(k == 0),
                    stop=(k == NT - 1),
                )
            nc.vector.tensor_copy(out=psb_tiles[0][0:2, D + 2 : PAD], in_=p22)

            # ---- all-reduce P across the 8 cores ----------------------------
            if use_collective:
                cc_in = dram.tile([512, PAD], f32, name="cc_in")
                cc_out = dram.tile([512, PAD], f32, name="cc_out", addr_space="Shared")
                for mt in range(4):
                    nc.sync.dma_start(
                        out=cc_in[mt * 128 : (mt + 1) * 128, :], in_=psb_tiles[mt]
                    )
                nc.gpsimd.collective_compute(
                    "AllReduce",
                    mybir.AluOpType.add,
                    replica_groups=[list(range(N_CORES))],
                    ins=[cc_in.opt()],
                    outs=[cc_out.opt()],
                )
                q_tiles = []
                for mt in range(4):
                    qt = work.tile([128, PAD], f32, name=f"q{mt}", tag="q")
                    nc.sync.dma_start(
                        out=qt, in_=cc_out[mt * 128 : (mt + 1) * 128, :]
                    )
                    q_tiles.append(qt)
            else:
                q_tiles = psb_tiles

            # expose P (summed if collective, else the local partial)
            for mt in range(4):
                nc.sync.dma_start(
                    out=out_p[mt * 128 : (mt + 1) * 128, :], in_=q_tiles[mt]
                )

            # ---- weighted sum of squares ------------------------------------
            # acc columns: 0..3 = ff blocks (+1), 4..7 = fc blocks (-2),
            # 8 = cc block (+1); weights (incl. 1/bs^2) come in via wvec.
            acc = res.tile([128, 16], f32, name="acc")
            nc.vector.memset(acc, 0.0)
            for mt in range(4):
                qt = q_tiles[mt]
                sqo = scratch.tile([128, D], f32, name=f"sqo{mt}", tag="sq")
                nc.scalar.activation(
                    out=sqo,
                    in_=qt[:, 0:D],
                    func=AF.Square,
                    accum_out=acc[:, mt : mt + 1],
                )
                sq2 = small.tile([128, 2], f32, name=f"sqs{mt}", tag="sq2")
                nc.scalar.activation(
                    out=sq2,
                    in_=qt[:, D : D + 2],
                    func=AF.Square,
                    accum_out=acc[:, 4 + mt : 5 + mt],
                )
            sq3 = small.tile([2, 2], f32, name="sq3", tag="sq3")
            nc.scalar.activation(
                out=sq3,
                in_=q_tiles[0][0:2, D + 2 : D + 4],
                func=AF.Square,
                accum_out=acc[0:2, 8:9],
            )

            # reduce acc over partitions with a ones-matvec, apply weights
            ones = res.tile([128, 1], f32, name="ones")
            nc.vector.memset(ones, 1.0)
            accsum = pfin.tile([1, 16], f32, name="accsum")
            nc.tensor.matmul(accsum, lhsT=ones, rhs=acc, start=True, stop=True)
            wsum = small.tile([1, 16], f32, name="wsum", tag="wsum")
            nc.vector.tensor_mul(wsum, accsum, wv)
            fin = small.tile([1, 1], f32, name="fin", tag="fin")
            nc.vector.reduce_sum(fin, wsum, axis=mybir.AxisListType.X)
            nc.sync.dma_start(out=out_num, in_=fin)

    nc.compile()
    return nc


def _get_module(use_collective):
    key = ("mod", use_collective)
    if key not in _CACHE:
        _CACHE[key] = _build_module(use_collective)
    return _CACHE[key]


def _weights():
    w = np.zeros((1, 16), dtype=np.float32)
    w[0, 0:4] = 1.0
    w[0, 4:8] = -2.0
    w[0, 8] = 1.0
    return w / np.float32(BS) / np.float32(BS)


def _run(batch_label, batch_feat, use_collective=USE_COLLECTIVE, trace=False):
    from concourse.bass_utils import run_bass_kernel_spmd

    nc = _get_module(use_collective)

    lab_f = np.ascontiguousarray(np.asarray(batch_label).astype(np.float32))
    feat = np.ascontiguousarray(np.asarray(batch_feat, dtype=np.float32))
    assert feat.shape == (BS, D) and lab_f.shape == (BS,)
    wv = _weights()

    in_maps = []
    for m in range(N_CORES):
        lab_m = lab_f[m * LOCAL : (m + 1) * LOCAL]
        in_maps.append(
            {
                "feat_loc": np.ascontiguousarray(feat[m * LOCAL : (m + 1) * LOCAL]),
                # [128, NT]: column t = labels of local batch rows t*128..t*128+127
                "lab_grid": np.ascontiguousarray(lab_m.reshape(NT, 128).T),
                "wvec": wv,
            }
        )

    res = run_bass_kernel_spmd(
        nc, in_maps, core_ids=list(range(N_CORES)), trace=trace
    )
    return res


def kernel(batch_label, batch_feat):
    res = _run(batch_label, batch_feat)
    if USE_COLLECTIVE:
        loss = res.results[0]["out_num"][0, 0]
    else:
        # host-side all-reduce of the partial P matrices (debug path)
        P = np.zeros((512, PAD), dtype=np.float64)
        for r in res.results:
            P += r["out_p"]
        s = (
            (P[:, 0:D] ** 2).sum()
            - 2.0 * (P[:, D : D + 2] ** 2).sum()
            + (P[0:2, D + 2 : D + 4] ** 2).sum()
        )
        loss = np.float32(s / BS / BS)
    return np.float32(loss)
